# revision 1
# baseline (speedup 1.0000x reference)
"""Game-of-Life CNN (3x3 circular conv + double-heaviside) on 8 trn2 cores.

Strategy:
  - Data-parallel over batch: 16 images -> 8 cores x 2 images. No halo
    exchange needed (each image is independent).
  - Host pre-pads each image with its circular halo -> [H+2, W+2], so
    every device tile load is a single contiguous DMA and no wrap
    handling is needed on device.
  - Per image: 17 row-tiles. Tile t loads padded rows r0..r0+127
    (126 output rows + halo) as bf16 -> xp [128, 2050].
  - Conv c (stencil [[2,2,2],[2,1,2],[2,2,2]]): columns 0 and 2 of the
    stencil are identical, so
      c = T2 @ (x_left + x_right) + W1 @ x_center
    where T2/W1 are 128x128 banded matrices applied along the row
    (partition) axis by the TensorEngine, and the (left+right) sum is
    one VectorE bf16 add (2x mode) along the free axis.
  - Threshold: out = (y>=4.5)-(y>=7.5), y = c + bias. With binary input
    c is integer-valued, so out == (|c - (6-bias)| <= 1.5) exactly.
    ScalarE computes Abs(psum - (6-bias)) -> bf16; VectorE compares
    <= 1.5 (4x mode) -> uint8; DMA out; host casts to float32.

IO in bf16 (exact for 0/1 input) and uint8 output to halve/quarter HBM
traffic vs f32 (memory-bound problem).
"""

import numpy as np
import ml_dtypes

import concourse.bass as bass
import concourse.bacc as bacc
import concourse.mybir as mybir
from concourse import tile
from concourse.bass_utils import run_bass_kernel_spmd

B, H, W = 16, 2048, 2048
NCORES = 8
IPC = B // NCORES          # images per core
HP, WP = H + 2, W + 2      # padded dims
ROWS_OUT = 126             # output rows per tile
NT = (H + ROWS_OUT - 1) // ROWS_OUT  # 17 tiles (16 x 126 + 1 x 32)
BF16 = mybir.dt.bfloat16
U8 = mybir.dt.uint8
F32 = mybir.dt.float32


def _build_nc(m_const: float):
    nc = bacc.Bacc()
    x = nc.dram_tensor("x", [IPC * HP, WP], BF16, kind="ExternalInput")
    # [T2 | W1] side by side
    wmat = nc.dram_tensor("wmat", [128, 256], BF16, kind="ExternalInput")
    y = nc.dram_tensor("y", [IPC * H, W], U8, kind="ExternalOutput")

    with tile.TileContext(nc) as tc:
        with (
            tc.tile_pool(name="const", bufs=1) as cpool,
            tc.tile_pool(name="xp", bufs=6) as xpool,
            tc.tile_pool(name="s", bufs=4) as spool,
            tc.tile_pool(name="t", bufs=4) as tpool,
            tc.tile_pool(name="o", bufs=6) as opool,
            tc.tile_pool(name="ps", bufs=2, space="PSUM") as pspool,
        ):
            wsb = cpool.tile([128, 256], BF16)
            nc.sync.dma_start(out=wsb[:, :], in_=wmat[:, :])
            T2 = wsb[:, 0:128]
            W1 = wsb[:, 128:256]
            bias_sb = cpool.tile([128, 1], F32, tag="bias")
            nc.vector.memset(bias_sb[:, :], -m_const)

            for img in range(IPC):
                for t in range(NT):
                    r0 = t * ROWS_OUT
                    n_out = min(ROWS_OUT, H - r0)
                    n_in = n_out + 2

                    # xp partition k = image row r0-1+k, col j = img col j-1
                    xp = xpool.tile([128, WP], BF16, tag="xp")
                    nc.sync.dma_start(
                        out=xp[0:n_in, :],
                        in_=x[img * HP + r0 : img * HP + r0 + n_in, :])

                    # s = x_left + x_right
                    s = spool.tile([128, W], BF16, tag="s")
                    nc.vector.tensor_add(
                        s[0:n_in, :], xp[0:n_in, 0:W], xp[0:n_in, 2:WP])

                    ps = pspool.tile([128, W], F32, tag="ps")
                    # weight-major order: keeps LDWEIGHTS count low
                    for st in range(4):
                        c0 = st * 512
                        nc.tensor.matmul(
                            ps[0:n_in, c0 : c0 + 512],
                            lhsT=T2[0:n_in, 0:n_in],
                            rhs=s[0:n_in, c0 : c0 + 512],
                            start=True, stop=False)
                    for st in range(4):
                        c0 = st * 512
                        nc.tensor.matmul(
                            ps[0:n_in, c0 : c0 + 512],
                            lhsT=W1[0:n_in, 0:n_in],
                            rhs=xp[0:n_in, c0 + 1 : c0 + 513],
                            start=False, stop=True)

                    # t = |c - (6 - bias)|
                    tt = tpool.tile([128, W], BF16, tag="t")
                    nc.scalar.activation(
                        tt[0:n_out, :], ps[0:n_out, :],
                        mybir.ActivationFunctionType.Abs,
                        bias=bias_sb[0:n_out, :], scale=1.0)

                    # o = (t <= 1.5) as u8
                    o = opool.tile([128, W], U8, tag="o")
                    nc.vector.tensor_scalar(
                        o[0:n_out, :], tt[0:n_out, :],
                        1.5, None, mybir.AluOpType.is_le)

                    nc.sync.dma_start(
                        out=y[img * H + r0 : img * H + r0 + n_out, :],
                        in_=o[0:n_out, :])
    nc.finalize()
    return nc


def _weight_mats(wk: np.ndarray) -> np.ndarray:
    """Build [128, 256] = [T2 | W1] banded matrices from 3x3 stencil."""
    assert np.array_equal(wk[:, 0], wk[:, 2]), "stencil columns 0/2 must match"
    # psum partition i = image row r0+i needs xp partitions k = i..i+2
    # (xp partition k = image row r0-1+k), weight wk[k-i, col].
    T2 = np.zeros((128, 128), np.float32)
    W1 = np.zeros((128, 128), np.float32)
    for k in range(128):
        for i in range(max(0, k - 2), k + 1):
            T2[k, i] = wk[k - i, 0]
            W1[k, i] = wk[k - i, 1]
    return np.hstack([T2, W1]).astype(ml_dtypes.bfloat16)


def _pad_circular(xb: np.ndarray) -> np.ndarray:
    """[B, H, W] -> [B, H+2, W+2] with circular halo."""
    xp = np.empty((xb.shape[0], HP, WP), xb.dtype)
    xp[:, 1 : H + 1, 1 : W + 1] = xb
    xp[:, 0, 1 : W + 1] = xb[:, H - 1]
    xp[:, H + 1, 1 : W + 1] = xb[:, 0]
    xp[:, :, 0] = xp[:, :, W]
    xp[:, :, W + 1] = xp[:, :, 1]
    return xp


def _run(inputs, trace=False, **kw):
    x = np.asarray(inputs["x"])
    wk = np.asarray(
        inputs.get("kernel",
                   np.array([[2., 2., 2.], [2., 1., 2.], [2., 2., 2.]]))
    ).reshape(3, 3).astype(np.float32)
    bias = float(np.asarray(inputs.get("bias", np.zeros(1))).reshape(-1)[0])
    m_const = 6.0 - bias  # midpoint of [4.5-bias, 7.5-bias]

    nc = _build_nc(m_const)
    wmat = _weight_mats(wk)
    xb = _pad_circular(x.reshape(B, H, W).astype(ml_dtypes.bfloat16))
    in_maps = [
        {"x": xb[c * IPC : (c + 1) * IPC].reshape(IPC * HP, WP), "wmat": wmat}
        for c in range(NCORES)
    ]
    res = run_bass_kernel_spmd(nc, in_maps, core_ids=list(range(NCORES)),
                               trace=trace, **kw)
    out = np.empty((B, 1, H, W), np.float32)
    for c in range(NCORES):
        out[c * IPC : (c + 1) * IPC, 0] = (
            res.results[c]["y"].reshape(IPC, H, W).astype(np.float32))
    return out, res


def kernel(**inputs) -> np.ndarray:
    out, _ = _run(inputs, trace=False)
    return out

